# revision 32
# baseline (speedup 1.0000x reference)
"""Single-level 2D Haar DWT (periodization mode) on Trainium2.

Input x: (8, 512, 512, 16) fp32 NHWC. Output: (LL, LH, HL, HH), each
(8, 256, 256, 16) fp32 — +/- combinations of each 2x2 spatial block,
scaled by 0.5.

Sharding: pure data parallel — one batch sample per NeuronCore (8 cores).

All device I/O is fp16 (host casts; the x0.5 subband scale is applied
during the host-side fp16 -> fp32 upcast): 8.4 MB in + 8.4 MB out per
core.

DMA model measured on this part (SPMD, all 8 cores streaming):
  - descriptors from all queues funnel into 16 per-engine FIFOs and
    are processed in ARRIVAL order at ~25 GB/s/engine (~425 GB/s
    aggregate = this core's HBM share), so the GLOBAL issue order
    is the transfer schedule;
  - SWDGE (GpSimd Q0) generates descriptors in bulk; each HWDGE
    ring (SP Q1 / ACT Q10) trickles at ~28 ns/descriptor — an HWDGE
    load issued BEFORE the SWDGE flood lands ~11-13 us, later ones
    crawl; never head an HWDGE queue with the 256 B-row weight load
    (128 tiny descriptors = 3.6 us of generator time);
  - at most ~8 outstanding SWDGE dma_starts before a framework
    DRAIN stalls the GpSimd stream (later store issues just wait);
  - a dma_start on a backed-up queue stalls its ENGINE 1-2 us, so
    all ScalarE copies are emitted at higher priority than any
    store issue.

The critical chain is VectorE (30.5 us of butterfly work): B0.top
is therefore the first-landing tile (Q1 head), and kc0 (PE's first
food) rides Q10's head. Everything else streams on Q0 in
consumption order; B1 lands last and fills VectorE's tail.

Work split by W-halves across two compute paths:

Path A (W cols 0:4096) — TensorE + ScalarE + VectorE, 8 units of
  128 rows x 2048 cols:
  - TensorE: row (H) butterfly as matmul with a fixed 128x128 +/-1
    fp16 weight (PSUM p0:63 = top+bot, p64:127 = top-bot);
  - ScalarE (ACT): PSUM -> SBUF copy, fp32 -> fp16;
  - VectorE: column (W) butterfly into per-kc [128, 2048] sum/diff
    tiles -> 4 merged [64 row, 4 KB] stores per kc.

Path B (W cols 4096:8192) — VectorE only, 2 units of 128 row-pairs:
  8-op butterfly (W-first: the first two mids need only `top`).

All tiles are resident in SBUF (~193 KB/partition, no reuse).
Bacc is built with num_devices=1: no collectives needed.
"""

import sys

if "/opt/trn_rl_repo" not in sys.path:
    sys.path.insert(0, "/opt/trn_rl_repo")

import numpy as np

B, H, W, C = 8, 512, 512, 16
N_CORES = 8
HO, WO = H // 2, W // 2  # 256, 256
ROW = W * C  # 8192 elements per input row
OROW = WO * C  # 4096 elements per output row

_CACHE = {}


def _haar_weight():
    """lhsT [k, m]: matmul computes out[m, n] = sum_k w[k, m] x[k, n]."""
    w = np.zeros((128, 128), dtype=np.float16)
    for m in range(64):
        w[2 * m, m] = 1.0
        w[2 * m + 1, m] = 1.0
        w[2 * m, 64 + m] = 1.0
        w[2 * m + 1, 64 + m] = -1.0
    return w


def _build():
    import concourse.bacc as bacc
    import concourse.mybir as mybir
    import concourse.tile as tile

    fp32 = mybir.dt.float32
    fp16 = mybir.dt.float16

    nc = bacc.Bacc(
        "TRN2", target_bir_lowering=False, debug=False, num_devices=1
    )
    x = nc.dram_tensor("x", (H, ROW), fp16, kind="ExternalInput")
    wdram = nc.dram_tensor("w", (128, 128), fp16, kind="ExternalInput")
    outs = {
        name: nc.dram_tensor(name, (HO, OROW), fp16, kind="ExternalOutput")
        for name in ("LL", "LH", "HL", "HH")
    }

    xq = x.rearrange("(q t) m -> q t m", t=2)  # [pair, row-parity, cols]

    HALF = ROW // 2  # 4096: A path covers cols 0:HALF, B path HALF:ROW
    AW = 2048  # A unit width (input cols); 4 matmuls of 512
    MM_N = 512  # one fp32 matmul / PSUM bank

    with tile.TileContext(nc) as tc:
        with (
            tc.tile_pool(name="main", bufs=1) as pool,
            tc.tile_pool(name="psum", bufs=2, space="PSUM") as psum,
        ):
            wt = pool.tile([128, 128], fp16, tag="wt")

            # ---- tiles ----
            tops = {}
            bots = {}
            for pc in range(2):
                tops[pc] = pool.tile(
                    [128, HALF], fp16, tag=f"top{pc}", name=f"top{pc}"
                )
                bots[pc] = pool.tile(
                    [128, HALF], fp16, tag=f"bot{pc}", name=f"bot{pc}"
                )
            # kc=0: two half tiles (Q10's first instructions);
            # kc=1,2,3: one [128, 4096] tile each on SWDGE.
            xth = {}
            for g in range(2):
                xth[g] = pool.tile(
                    [128, AW], fp16, tag=f"xt0{g}", name=f"xt0{g}"
                )
            xtf = {}
            for kc in range(1, 4):
                xtf[kc] = pool.tile(
                    [128, HALF], fp16, tag=f"xt{kc}", name=f"xt{kc}"
                )

            def a_src(kc, g):
                if kc == 0:
                    return xth[g][:]
                return xtf[kc][:, g * AW : (g + 1) * AW]

            # ---- loads (global arrival order = issue order) ----
            # Exactly ONE head load per HWDGE ring: its descriptors
            # finish generating (~28 ns each) before the SWDGE flood
            # arrives (~11.8 us), so it lands ~13 us. A second head
            # load would generate too late and get jumped by the flood.
            # Q1 head: B0.top (VectorE's first food).
            nc.sync.dma_start(tops[0][:], xq[slice(0, 128), 0, HALF:ROW])
            # Q10 head: kc0 g0 (PE's first food).
            nc.scalar.dma_start(xth[0][:], x[0:128, 0:AW])
            # Q0 (SWDGE): wt (tiny, bulk-generated), then consumption
            # order — kc0g1, B0.bot, kc1..3, B1.
            nc.gpsimd.dma_start(wt[:], wdram[:])
            nc.gpsimd.dma_start(bots[0][:], xq[slice(0, 128), 1, HALF:ROW])
            nc.gpsimd.dma_start(xth[1][:], x[0:128, AW : 2 * AW])
            for kc in range(1, 4):
                nc.gpsimd.dma_start(
                    xtf[kc][:], x[kc * 128 : (kc + 1) * 128, 0:HALF]
                )
            nc.gpsimd.dma_start(tops[1][:], xq[slice(128, 256), 0, HALF:ROW])
            nc.gpsimd.dma_start(bots[1][:], xq[slice(128, 256), 1, HALF:ROW])

            # ---- B0 mids: highest DVE priority ----
            mids = {}
            for pc in range(2):
                for mt in ("t1", "t2", "u1", "u2"):
                    mids[(pc, mt)] = pool.tile(
                        [128, HALF // 2],
                        fp16,
                        tag=f"m{mt}{pc}",
                        name=f"m{mt}{pc}",
                    )

            def emit_b_mids(pc):
                tv = tops[pc][:].rearrange("p (w u c) -> p w u c", u=2, c=C)
                bv = bots[pc][:].rearrange("p (w u c) -> p w u c", u=2, c=C)
                a, b = tv[:, :, 0, :], tv[:, :, 1, :]
                c_, d = bv[:, :, 0, :], bv[:, :, 1, :]
                m = lambda mt: mids[(pc, mt)][:].rearrange(
                    "p (w c) -> p w c", c=C
                )
                # top-only ops first: they unblock as soon as `top` lands
                nc.vector.tensor_add(m("t1"), a, b)
                nc.vector.tensor_sub(m("u1"), a, b)
                nc.vector.tensor_add(m("t2"), c_, d)
                nc.vector.tensor_sub(m("u2"), c_, d)

            emit_b_mids(0)

            # ---- A units in land order (no stores yet: ScalarE's
            # copies must outrank every store issue) ----
            sums = {}
            diffs = {}
            for kc in range(4):
                sums[kc] = pool.tile(
                    [128, AW], fp16, tag=f"s{kc}", name=f"s{kc}"
                )
                diffs[kc] = pool.tile(
                    [128, AW], fp16, tag=f"d{kc}", name=f"d{kc}"
                )

            def emit_a_unit(kc, g):
                xt = a_src(kc, g)
                ps = psum.tile([128, AW], fp32)
                for j in range(AW // MM_N):
                    lo = j * MM_N
                    nc.tensor.matmul(
                        ps[:, lo : lo + MM_N],
                        wt[:],
                        xt[:, lo : lo + MM_N],
                        start=True,
                        stop=True,
                    )
                sb = pool.tile([128, AW], fp16, tag=f"sb{kc}{g}")
                nc.scalar.copy(sb[:], ps[:])  # ACT: PSUM -> SBUF, fp32->fp16
                sv_in = sb[:].rearrange("p (w u c) -> p w u c", u=2, c=C)
                ev, od = sv_in[:, :, 0, :], sv_in[:, :, 1, :]
                half = slice(g * (AW // 2), (g + 1) * (AW // 2))
                sv = sums[kc][:, half].rearrange("p (w c) -> p w c", c=C)
                dv = diffs[kc][:, half].rearrange("p (w c) -> p w c", c=C)
                nc.vector.tensor_add(sv, ev, od)
                nc.vector.tensor_sub(dv, ev, od)

            for kc in range(4):
                emit_a_unit(kc, 0)
                emit_a_unit(kc, 1)

            # ---- A stores (after all copies in priority) ----
            def emit_a_stores(kc, rings):
                rs = slice(kc * 64, (kc + 1) * 64)
                cols = slice(0, AW)
                r0, r1, r2, r3 = rings
                r0.dma_start(outs["LL"][rs, cols], sums[kc][0:64, :])
                r1.dma_start(outs["HL"][rs, cols], sums[kc][64:128, :])
                r2.dma_start(outs["LH"][rs, cols], diffs[kc][0:64, :])
                r3.dma_start(outs["HH"][rs, cols], diffs[kc][64:128, :])

            emit_a_stores(0, (nc.sync,) * 4)
            emit_a_stores(1, (nc.scalar,) * 4)
            emit_a_stores(2, (nc.sync,) * 4)
            emit_a_stores(3, (nc.gpsimd,) * 4)

            # ---- B outs: fill DVE gaps ----
            def emit_b_outs(pc, rings):
                qs = slice(pc * 128, (pc + 1) * 128)
                oc = slice(HALF // 2, OROW)
                WQ = HALF // (2 * C)
                for (name, i0, i1, op), ring in zip(
                    (
                        ("LL", "t1", "t2", "add"),
                        ("HL", "t1", "t2", "sub"),
                        ("LH", "u1", "u2", "add"),
                        ("HH", "u1", "u2", "sub"),
                    ),
                    rings,
                ):
                    ot = pool.tile([128, WQ, C], fp16, tag=f"o{name}{pc}")
                    a0 = mids[(pc, i0)][:].rearrange("p (w c) -> p w c", c=C)
                    a1 = mids[(pc, i1)][:].rearrange("p (w c) -> p w c", c=C)
                    if op == "add":
                        nc.vector.tensor_add(ot[:], a0, a1)
                    else:
                        nc.vector.tensor_sub(ot[:], a0, a1)
                    ring.dma_start(
                        outs[name][qs, oc],
                        ot[:].rearrange("p w c -> p (w c)"),
                    )

            # B0: LL/HL on HWDGE rings, LH/HH ride Q0 after its loads;
            # B1: LL/HL on HWDGE, LH/HH on Q0's free tail.
            emit_b_outs(0, (nc.sync, nc.scalar, nc.sync, nc.scalar))
            emit_b_mids(1)
            emit_b_outs(1, (nc.gpsimd, nc.gpsimd, nc.gpsimd, nc.gpsimd))

    nc.compile()
    return nc


def _get_nc():
    if "nc" not in _CACHE:
        _CACHE["nc"] = _build()
    return _CACHE["nc"]


def _in_maps(x):
    w = _haar_weight()
    xh = np.asarray(x, dtype=np.float16)
    return [
        {"x": np.ascontiguousarray(xh[i].reshape(H, ROW)), "w": w}
        for i in range(B)
    ]


def kernel(x):
    from concourse.bass_utils import run_bass_kernel_spmd

    x = np.asarray(x, dtype=np.float32)
    assert x.shape == (B, H, W, C), x.shape

    nc = _get_nc()
    try:
        res = run_bass_kernel_spmd(nc, _in_maps(x), list(range(N_CORES)))
    except Exception:
        # transient NRT device errors have been observed right after
        # compile; one retry has always succeeded
        res = run_bass_kernel_spmd(nc, _in_maps(x), list(range(N_CORES)))

    out = []
    for name in ("LL", "LH", "HL", "HH"):
        sub = np.stack(
            [res.results[i][name].reshape(HO, WO, C) for i in range(B)],
            axis=0,
        )
        out.append(sub.astype(np.float32) * np.float32(0.5))
    return tuple(out)


# revision 33
# speedup vs baseline: 1.0121x; 1.0121x over previous
"""Single-level 2D Haar DWT (periodization mode) on Trainium2.

Input x: (8, 512, 512, 16) fp32 NHWC. Output: (LL, LH, HL, HH), each
(8, 256, 256, 16) fp32 — +/- combinations of each 2x2 spatial block,
scaled by 0.5.

Sharding: pure data parallel — one batch sample per NeuronCore (8 cores).

All device I/O is fp16 (host casts; the x0.5 subband scale is applied
during the host-side fp16 -> fp32 upcast): 8.4 MB in + 8.4 MB out per
core.

DMA model measured on this part (SPMD, all 8 cores streaming):
  - descriptors from all queues funnel into 16 per-engine FIFOs and
    are processed in ARRIVAL order at ~25 GB/s/engine (~425 GB/s
    aggregate = this core's HBM share), so the GLOBAL issue order
    is the transfer schedule;
  - SWDGE (GpSimd Q0) generates descriptors in bulk; each HWDGE
    ring (SP Q1 / ACT Q10) trickles at ~28 ns/descriptor — an HWDGE
    load issued BEFORE the SWDGE flood lands ~11-13 us, later ones
    crawl; never head an HWDGE queue with the 256 B-row weight load
    (128 tiny descriptors = 3.6 us of generator time);
  - at most ~8 outstanding SWDGE dma_starts before a framework
    DRAIN stalls the GpSimd stream (later store issues just wait);
  - a dma_start on a backed-up queue stalls its ENGINE 1-2 us, so
    all ScalarE copies are emitted at higher priority than any
    store issue.

The critical chain is VectorE (30.5 us of butterfly work): B0.top
is therefore the first-landing tile (Q1 head), and kc0 (PE's first
food) rides Q10's head. Everything else streams on Q0 in
consumption order; B1 lands last and fills VectorE's tail.

Work split by W-halves across two compute paths:

Path A (W cols 0:4096) — TensorE + ScalarE + VectorE, 8 units of
  128 rows x 2048 cols:
  - TensorE: row (H) butterfly as matmul with a fixed 128x128 +/-1
    fp16 weight (PSUM p0:63 = top+bot, p64:127 = top-bot);
  - ScalarE (ACT): PSUM -> SBUF copy, fp32 -> fp16;
  - VectorE: column (W) butterfly into per-kc [128, 2048] sum/diff
    tiles -> 4 merged [64 row, 4 KB] stores per kc.

Path B (W cols 4096:8192) — VectorE only, 2 units of 128 row-pairs:
  8-op butterfly (W-first: the first two mids need only `top`).

All tiles are resident in SBUF (~193 KB/partition, no reuse).
Bacc is built with num_devices=1: no collectives needed.
"""

import sys

if "/opt/trn_rl_repo" not in sys.path:
    sys.path.insert(0, "/opt/trn_rl_repo")

import numpy as np

B, H, W, C = 8, 512, 512, 16
N_CORES = 8
HO, WO = H // 2, W // 2  # 256, 256
ROW = W * C  # 8192 elements per input row
OROW = WO * C  # 4096 elements per output row

_CACHE = {}


def _haar_weight():
    """lhsT [k, m]: matmul computes out[m, n] = sum_k w[k, m] x[k, n]."""
    w = np.zeros((128, 128), dtype=np.float16)
    for m in range(64):
        w[2 * m, m] = 1.0
        w[2 * m + 1, m] = 1.0
        w[2 * m, 64 + m] = 1.0
        w[2 * m + 1, 64 + m] = -1.0
    return w


def _build():
    import concourse.bacc as bacc
    import concourse.mybir as mybir
    import concourse.tile as tile

    fp32 = mybir.dt.float32
    fp16 = mybir.dt.float16

    nc = bacc.Bacc(
        "TRN2", target_bir_lowering=False, debug=False, num_devices=1
    )
    x = nc.dram_tensor("x", (H, ROW), fp16, kind="ExternalInput")
    wdram = nc.dram_tensor("w", (128, 128), fp16, kind="ExternalInput")
    outs = {
        name: nc.dram_tensor(name, (HO, OROW), fp16, kind="ExternalOutput")
        for name in ("LL", "LH", "HL", "HH")
    }

    xq = x.rearrange("(q t) m -> q t m", t=2)  # [pair, row-parity, cols]

    HALF = ROW // 2  # 4096: A path covers cols 0:HALF, B path HALF:ROW
    AW = 2048  # A unit width (input cols); 4 matmuls of 512
    MM_N = 512  # one fp32 matmul / PSUM bank

    with tile.TileContext(nc) as tc:
        with (
            tc.tile_pool(name="main", bufs=1) as pool,
            tc.tile_pool(name="psum", bufs=2, space="PSUM") as psum,
        ):
            wt = pool.tile([128, 128], fp16, tag="wt")

            # ---- tiles ----
            tops = {}
            bots = {}
            for pc in range(2):
                tops[pc] = pool.tile(
                    [128, HALF], fp16, tag=f"top{pc}", name=f"top{pc}"
                )
                bots[pc] = pool.tile(
                    [128, HALF], fp16, tag=f"bot{pc}", name=f"bot{pc}"
                )
            # kc=0: two half tiles (Q10's first instructions);
            # kc=1,2,3: one [128, 4096] tile each on SWDGE.
            xth = {}
            for g in range(2):
                xth[g] = pool.tile(
                    [128, AW], fp16, tag=f"xt0{g}", name=f"xt0{g}"
                )
            xtf = {}
            for kc in range(1, 4):
                xtf[kc] = pool.tile(
                    [128, HALF], fp16, tag=f"xt{kc}", name=f"xt{kc}"
                )

            def a_src(kc, g):
                if kc == 0:
                    return xth[g][:]
                return xtf[kc][:, g * AW : (g + 1) * AW]

            # ---- loads (global arrival order = issue order) ----
            # Exactly ONE head load per HWDGE ring: its descriptors
            # finish generating (~28 ns each) before the SWDGE flood
            # arrives (~11.8 us), so it lands ~13 us. A second head
            # load would generate too late and get jumped by the flood.
            # Q1 head: B0.top (VectorE's first food).
            nc.sync.dma_start(tops[0][:], xq[slice(0, 128), 0, HALF:ROW])
            # Q10 head: kc0 g0 (PE's first food).
            nc.scalar.dma_start(xth[0][:], x[0:128, 0:AW])
            # Q0 (SWDGE): wt (tiny, bulk-generated), then consumption
            # order — kc0g1, B0.bot, kc1..3, B1.
            nc.gpsimd.dma_start(wt[:], wdram[:])
            nc.gpsimd.dma_start(bots[0][:], xq[slice(0, 128), 1, HALF:ROW])
            nc.gpsimd.dma_start(xth[1][:], x[0:128, AW : 2 * AW])
            for kc in range(1, 4):
                nc.gpsimd.dma_start(
                    xtf[kc][:], x[kc * 128 : (kc + 1) * 128, 0:HALF]
                )
            nc.gpsimd.dma_start(tops[1][:], xq[slice(128, 256), 0, HALF:ROW])
            nc.gpsimd.dma_start(bots[1][:], xq[slice(128, 256), 1, HALF:ROW])

            # ---- B0 mids: highest DVE priority ----
            mids = {}
            for pc in range(2):
                for mt in ("t1", "t2", "u1", "u2"):
                    mids[(pc, mt)] = pool.tile(
                        [128, HALF // 2],
                        fp16,
                        tag=f"m{mt}{pc}",
                        name=f"m{mt}{pc}",
                    )

            def emit_b_mids(pc):
                tv = tops[pc][:].rearrange("p (w u c) -> p w u c", u=2, c=C)
                bv = bots[pc][:].rearrange("p (w u c) -> p w u c", u=2, c=C)
                a, b = tv[:, :, 0, :], tv[:, :, 1, :]
                c_, d = bv[:, :, 0, :], bv[:, :, 1, :]
                m = lambda mt: mids[(pc, mt)][:].rearrange(
                    "p (w c) -> p w c", c=C
                )
                # top-only ops first: they unblock as soon as `top` lands
                nc.vector.tensor_add(m("t1"), a, b)
                nc.vector.tensor_sub(m("u1"), a, b)
                nc.vector.tensor_add(m("t2"), c_, d)
                nc.vector.tensor_sub(m("u2"), c_, d)

            emit_b_mids(0)

            # ---- A units in land order (no stores yet: ScalarE's
            # copies must outrank every store issue) ----
            sums = {}
            diffs = {}
            for kc in range(4):
                sums[kc] = pool.tile(
                    [128, AW], fp16, tag=f"s{kc}", name=f"s{kc}"
                )
                diffs[kc] = pool.tile(
                    [128, AW], fp16, tag=f"d{kc}", name=f"d{kc}"
                )

            def emit_a_unit(kc, g):
                xt = a_src(kc, g)
                ps = psum.tile([128, AW], fp32)
                for j in range(AW // MM_N):
                    lo = j * MM_N
                    nc.tensor.matmul(
                        ps[:, lo : lo + MM_N],
                        wt[:],
                        xt[:, lo : lo + MM_N],
                        start=True,
                        stop=True,
                    )
                sb = pool.tile([128, AW], fp16, tag=f"sb{kc}{g}")
                nc.scalar.copy(sb[:], ps[:])  # ACT: PSUM -> SBUF, fp32->fp16
                sv_in = sb[:].rearrange("p (w u c) -> p w u c", u=2, c=C)
                ev, od = sv_in[:, :, 0, :], sv_in[:, :, 1, :]
                half = slice(g * (AW // 2), (g + 1) * (AW // 2))
                sv = sums[kc][:, half].rearrange("p (w c) -> p w c", c=C)
                dv = diffs[kc][:, half].rearrange("p (w c) -> p w c", c=C)
                nc.vector.tensor_add(sv, ev, od)
                nc.vector.tensor_sub(dv, ev, od)

            for kc in range(4):
                emit_a_unit(kc, 0)
                emit_a_unit(kc, 1)

            # ---- A stores (after all copies in priority) ----
            def emit_a_stores(kc, rings):
                rs = slice(kc * 64, (kc + 1) * 64)
                cols = slice(0, AW)
                r0, r1, r2, r3 = rings
                r0.dma_start(outs["LL"][rs, cols], sums[kc][0:64, :])
                r1.dma_start(outs["HL"][rs, cols], sums[kc][64:128, :])
                r2.dma_start(outs["LH"][rs, cols], diffs[kc][0:64, :])
                r3.dma_start(outs["HH"][rs, cols], diffs[kc][64:128, :])

            emit_a_stores(0, (nc.sync,) * 4)
            emit_a_stores(1, (nc.scalar,) * 4)
            emit_a_stores(2, (nc.sync,) * 4)
            emit_a_stores(3, (nc.gpsimd,) * 4)

            # ---- B outs: fill DVE gaps ----
            def emit_b_outs(pc, rings):
                qs = slice(pc * 128, (pc + 1) * 128)
                oc = slice(HALF // 2, OROW)
                WQ = HALF // (2 * C)
                for (name, i0, i1, op), ring in zip(
                    (
                        ("LL", "t1", "t2", "add"),
                        ("HL", "t1", "t2", "sub"),
                        ("LH", "u1", "u2", "add"),
                        ("HH", "u1", "u2", "sub"),
                    ),
                    rings,
                ):
                    ot = pool.tile([128, WQ, C], fp16, tag=f"o{name}{pc}")
                    a0 = mids[(pc, i0)][:].rearrange("p (w c) -> p w c", c=C)
                    a1 = mids[(pc, i1)][:].rearrange("p (w c) -> p w c", c=C)
                    if op == "add":
                        nc.vector.tensor_add(ot[:], a0, a1)
                    else:
                        nc.vector.tensor_sub(ot[:], a0, a1)
                    ring.dma_start(
                        outs[name][qs, oc],
                        ot[:].rearrange("p w c -> p (w c)"),
                    )

            # B0: LL/HL on HWDGE rings, LH/HH ride Q0 after its loads;
            # B1: LL/HL on HWDGE, LH/HH on Q0's free tail.
            emit_b_outs(0, (nc.sync, nc.scalar, nc.sync, nc.scalar))
            emit_b_mids(1)
            emit_b_outs(1, (nc.gpsimd, nc.gpsimd, nc.sync, nc.scalar))

    nc.compile()
    return nc


def _get_nc():
    if "nc" not in _CACHE:
        _CACHE["nc"] = _build()
    return _CACHE["nc"]


def _in_maps(x):
    w = _haar_weight()
    xh = np.asarray(x, dtype=np.float16)
    return [
        {"x": np.ascontiguousarray(xh[i].reshape(H, ROW)), "w": w}
        for i in range(B)
    ]


def kernel(x):
    from concourse.bass_utils import run_bass_kernel_spmd

    x = np.asarray(x, dtype=np.float32)
    assert x.shape == (B, H, W, C), x.shape

    nc = _get_nc()
    try:
        res = run_bass_kernel_spmd(nc, _in_maps(x), list(range(N_CORES)))
    except Exception:
        # transient NRT device errors have been observed right after
        # compile; one retry has always succeeded
        res = run_bass_kernel_spmd(nc, _in_maps(x), list(range(N_CORES)))

    out = []
    for name in ("LL", "LH", "HL", "HH"):
        sub = np.stack(
            [res.results[i][name].reshape(HO, WO, C) for i in range(B)],
            axis=0,
        )
        out.append(sub.astype(np.float32) * np.float32(0.5))
    return tuple(out)


# revision 34
# speedup vs baseline: 1.0222x; 1.0100x over previous
"""Single-level 2D Haar DWT (periodization mode) on Trainium2.

Input x: (8, 512, 512, 16) fp32 NHWC. Output: (LL, LH, HL, HH), each
(8, 256, 256, 16) fp32 — +/- combinations of each 2x2 spatial block,
scaled by 0.5.

Sharding: pure data parallel — one batch sample per NeuronCore (8 cores).

All device I/O is fp16 (host casts; the x0.5 subband scale is applied
during the host-side fp16 -> fp32 upcast): 8.4 MB in + 8.4 MB out per
core.

DMA model measured on this part (SPMD, all 8 cores streaming):
  - descriptors from all queues funnel into 16 per-engine FIFOs and
    are processed in ARRIVAL order at ~25 GB/s/engine (~425 GB/s
    aggregate = this core's HBM share), so the GLOBAL issue order
    is the transfer schedule;
  - SWDGE (GpSimd Q0) generates descriptors in bulk; each HWDGE
    ring (SP Q1 / ACT Q10) trickles at ~28 ns/descriptor — an HWDGE
    load issued BEFORE the SWDGE flood lands ~11-13 us, later ones
    crawl; never head an HWDGE queue with the 256 B-row weight load
    (128 tiny descriptors = 3.6 us of generator time);
  - at most ~8 outstanding SWDGE dma_starts before a framework
    DRAIN stalls the GpSimd stream (later store issues just wait);
  - a dma_start on a backed-up queue stalls its ENGINE 1-2 us, so
    all ScalarE copies are emitted at higher priority than any
    store issue.

The critical chain is VectorE (30.5 us of butterfly work): B0.top
is therefore the first-landing tile (Q1 head), and kc0 (PE's first
food) rides Q10's head. Everything else streams on Q0 in
consumption order; B1 lands last and fills VectorE's tail.

Work split by W-halves across two compute paths:

Path A (W cols 0:4096) — TensorE + ScalarE + VectorE, 8 units of
  128 rows x 2048 cols:
  - TensorE: row (H) butterfly as matmul with a fixed 128x128 +/-1
    fp16 weight (PSUM p0:63 = top+bot, p64:127 = top-bot);
  - ScalarE (ACT): PSUM -> SBUF copy, fp32 -> fp16;
  - VectorE: column (W) butterfly into per-kc [128, 2048] sum/diff
    tiles -> 4 merged [64 row, 4 KB] stores per kc.

Path B (W cols 4096:8192) — VectorE only, 2 units of 128 row-pairs:
  8-op butterfly (W-first: the first two mids need only `top`).

All tiles are resident in SBUF (~193 KB/partition, no reuse).
Bacc is built with num_devices=1: no collectives needed.
"""

import sys

if "/opt/trn_rl_repo" not in sys.path:
    sys.path.insert(0, "/opt/trn_rl_repo")

import numpy as np

B, H, W, C = 8, 512, 512, 16
N_CORES = 8
HO, WO = H // 2, W // 2  # 256, 256
ROW = W * C  # 8192 elements per input row
OROW = WO * C  # 4096 elements per output row

_CACHE = {}


def _haar_weight():
    """lhsT [k, m]: matmul computes out[m, n] = sum_k w[k, m] x[k, n]."""
    w = np.zeros((128, 128), dtype=np.float16)
    for m in range(64):
        w[2 * m, m] = 1.0
        w[2 * m + 1, m] = 1.0
        w[2 * m, 64 + m] = 1.0
        w[2 * m + 1, 64 + m] = -1.0
    return w


def _build():
    import concourse.bacc as bacc
    import concourse.mybir as mybir
    import concourse.tile as tile

    fp32 = mybir.dt.float32
    fp16 = mybir.dt.float16

    nc = bacc.Bacc(
        "TRN2", target_bir_lowering=False, debug=False, num_devices=1
    )
    x = nc.dram_tensor("x", (H, ROW), fp16, kind="ExternalInput")
    wdram = nc.dram_tensor("w", (128, 128), fp16, kind="ExternalInput")
    outs = {
        name: nc.dram_tensor(name, (HO, OROW), fp16, kind="ExternalOutput")
        for name in ("LL", "LH", "HL", "HH")
    }

    xq = x.rearrange("(q t) m -> q t m", t=2)  # [pair, row-parity, cols]

    HALF = ROW // 2  # 4096: A path covers cols 0:HALF, B path HALF:ROW
    AW = 2048  # A unit width (input cols); 4 matmuls of 512
    MM_N = 512  # one fp32 matmul / PSUM bank

    with tile.TileContext(nc) as tc:
        with (
            tc.tile_pool(name="main", bufs=1) as pool,
            tc.tile_pool(name="psum", bufs=2, space="PSUM") as psum,
        ):
            wt = pool.tile([128, 128], fp16, tag="wt")

            # ---- tiles ----
            tops = {}
            bots = {}
            for pc in range(2):
                tops[pc] = pool.tile(
                    [128, HALF], fp16, tag=f"top{pc}", name=f"top{pc}"
                )
                bots[pc] = pool.tile(
                    [128, HALF], fp16, tag=f"bot{pc}", name=f"bot{pc}"
                )
            # kc=0: two half tiles (Q10's first instructions);
            # kc=1,2,3: one [128, 4096] tile each on SWDGE.
            xth = {}
            for g in range(2):
                xth[g] = pool.tile(
                    [128, AW], fp16, tag=f"xt0{g}", name=f"xt0{g}"
                )
            xtf = {}
            for kc in range(1, 4):
                xtf[kc] = pool.tile(
                    [128, HALF], fp16, tag=f"xt{kc}", name=f"xt{kc}"
                )

            def a_src(kc, g):
                if kc == 0:
                    return xth[g][:]
                return xtf[kc][:, g * AW : (g + 1) * AW]

            # ---- loads (global arrival order = issue order) ----
            # Exactly ONE head load per HWDGE ring: its descriptors
            # finish generating (~28 ns each) before the SWDGE flood
            # arrives (~11.8 us), so it lands ~13 us. A second head
            # load would generate too late and get jumped by the flood.
            # Q1 head: B0.top (VectorE's first food).
            nc.sync.dma_start(tops[0][:], xq[slice(0, 128), 0, HALF:ROW])
            # Q10 head: kc0 g0 (PE's first food).
            nc.scalar.dma_start(xth[0][:], x[0:128, 0:AW])
            # Q0 (SWDGE): wt (tiny, bulk-generated), then consumption
            # order — kc0g1, B0.bot, kc1..3, B1.
            nc.gpsimd.dma_start(wt[:], wdram[:])
            nc.gpsimd.dma_start(bots[0][:], xq[slice(0, 128), 1, HALF:ROW])
            nc.gpsimd.dma_start(xth[1][:], x[0:128, AW : 2 * AW])
            for kc in range(1, 4):
                nc.gpsimd.dma_start(
                    xtf[kc][:], x[kc * 128 : (kc + 1) * 128, 0:HALF]
                )
            nc.gpsimd.dma_start(tops[1][:], xq[slice(128, 256), 0, HALF:ROW])
            nc.gpsimd.dma_start(bots[1][:], xq[slice(128, 256), 1, HALF:ROW])

            # ---- B0 mids: highest DVE priority ----
            mids = {}
            for pc in range(2):
                for mt in ("t1", "t2", "u1", "u2"):
                    mids[(pc, mt)] = pool.tile(
                        [128, HALF // 2],
                        fp16,
                        tag=f"m{mt}{pc}",
                        name=f"m{mt}{pc}",
                    )

            def emit_b_mids(pc):
                tv = tops[pc][:].rearrange("p (w u c) -> p w u c", u=2, c=C)
                bv = bots[pc][:].rearrange("p (w u c) -> p w u c", u=2, c=C)
                a, b = tv[:, :, 0, :], tv[:, :, 1, :]
                c_, d = bv[:, :, 0, :], bv[:, :, 1, :]
                m = lambda mt: mids[(pc, mt)][:].rearrange(
                    "p (w c) -> p w c", c=C
                )
                # top-only ops first: they unblock as soon as `top` lands
                nc.vector.tensor_add(m("t1"), a, b)
                nc.vector.tensor_sub(m("u1"), a, b)
                nc.vector.tensor_add(m("t2"), c_, d)
                nc.vector.tensor_sub(m("u2"), c_, d)

            emit_b_mids(0)

            # ---- A units in land order (no stores yet: ScalarE's
            # copies must outrank every store issue) ----
            sums = {}
            diffs = {}
            for kc in range(4):
                sums[kc] = pool.tile(
                    [128, AW], fp16, tag=f"s{kc}", name=f"s{kc}"
                )
                diffs[kc] = pool.tile(
                    [128, AW], fp16, tag=f"d{kc}", name=f"d{kc}"
                )

            def emit_a_unit(kc, g):
                xt = a_src(kc, g)
                ps = psum.tile([128, AW], fp32)
                for j in range(AW // MM_N):
                    lo = j * MM_N
                    nc.tensor.matmul(
                        ps[:, lo : lo + MM_N],
                        wt[:],
                        xt[:, lo : lo + MM_N],
                        start=True,
                        stop=True,
                    )
                sb = pool.tile([128, AW], fp16, tag=f"sb{kc}{g}")
                nc.scalar.copy(sb[:], ps[:])  # ACT: PSUM -> SBUF, fp32->fp16
                sv_in = sb[:].rearrange("p (w u c) -> p w u c", u=2, c=C)
                ev, od = sv_in[:, :, 0, :], sv_in[:, :, 1, :]
                half = slice(g * (AW // 2), (g + 1) * (AW // 2))
                sv = sums[kc][:, half].rearrange("p (w c) -> p w c", c=C)
                dv = diffs[kc][:, half].rearrange("p (w c) -> p w c", c=C)
                nc.vector.tensor_add(sv, ev, od)
                nc.vector.tensor_sub(dv, ev, od)

            for kc in range(4):
                emit_a_unit(kc, 0)
                emit_a_unit(kc, 1)

            # ---- A stores (after all copies in priority) ----
            def emit_a_stores(kc, rings):
                rs = slice(kc * 64, (kc + 1) * 64)
                cols = slice(0, AW)
                r0, r1, r2, r3 = rings
                r0.dma_start(outs["LL"][rs, cols], sums[kc][0:64, :])
                r1.dma_start(outs["HL"][rs, cols], sums[kc][64:128, :])
                r2.dma_start(outs["LH"][rs, cols], diffs[kc][0:64, :])
                r3.dma_start(outs["HH"][rs, cols], diffs[kc][64:128, :])

            emit_a_stores(0, (nc.sync,) * 4)
            emit_a_stores(1, (nc.scalar,) * 4)
            emit_a_stores(2, (nc.sync,) * 4)
            emit_a_stores(3, (nc.gpsimd,) * 4)

            # ---- B outs: fill DVE gaps ----
            def emit_b_outs(pc, rings):
                qs = slice(pc * 128, (pc + 1) * 128)
                oc = slice(HALF // 2, OROW)
                WQ = HALF // (2 * C)
                for (name, i0, i1, op), ring in zip(
                    (
                        ("LL", "t1", "t2", "add"),
                        ("HL", "t1", "t2", "sub"),
                        ("LH", "u1", "u2", "add"),
                        ("HH", "u1", "u2", "sub"),
                    ),
                    rings,
                ):
                    ot = pool.tile([128, WQ, C], fp16, tag=f"o{name}{pc}")
                    a0 = mids[(pc, i0)][:].rearrange("p (w c) -> p w c", c=C)
                    a1 = mids[(pc, i1)][:].rearrange("p (w c) -> p w c", c=C)
                    if op == "add":
                        nc.vector.tensor_add(ot[:], a0, a1)
                    else:
                        nc.vector.tensor_sub(ot[:], a0, a1)
                    ring.dma_start(
                        outs[name][qs, oc],
                        ot[:].rearrange("p w c -> p (w c)"),
                    )

            # B0: LL/HL on HWDGE rings, LH/HH ride Q0 after its loads;
            # B1: LL/HL on HWDGE, LH/HH on Q0's free tail.
            emit_b_outs(0, (nc.sync, nc.scalar, nc.sync, nc.scalar))
            emit_b_mids(1)
            emit_b_outs(1, (nc.gpsimd, nc.gpsimd, nc.sync, nc.gpsimd))

    nc.compile()
    return nc


def _get_nc():
    if "nc" not in _CACHE:
        _CACHE["nc"] = _build()
    return _CACHE["nc"]


def _in_maps(x):
    w = _haar_weight()
    xh = np.asarray(x, dtype=np.float16)
    return [
        {"x": np.ascontiguousarray(xh[i].reshape(H, ROW)), "w": w}
        for i in range(B)
    ]


def kernel(x):
    from concourse.bass_utils import run_bass_kernel_spmd

    x = np.asarray(x, dtype=np.float32)
    assert x.shape == (B, H, W, C), x.shape

    nc = _get_nc()
    try:
        res = run_bass_kernel_spmd(nc, _in_maps(x), list(range(N_CORES)))
    except Exception:
        # transient NRT device errors have been observed right after
        # compile; one retry has always succeeded
        res = run_bass_kernel_spmd(nc, _in_maps(x), list(range(N_CORES)))

    out = []
    for name in ("LL", "LH", "HL", "HH"):
        sub = np.stack(
            [res.results[i][name].reshape(HO, WO, C) for i in range(B)],
            axis=0,
        )
        out.append(sub.astype(np.float32) * np.float32(0.5))
    return tuple(out)


# revision 35
# speedup vs baseline: 1.0386x; 1.0160x over previous
"""Single-level 2D Haar DWT (periodization mode) on Trainium2.

Input x: (8, 512, 512, 16) fp32 NHWC. Output: (LL, LH, HL, HH), each
(8, 256, 256, 16) fp32 — +/- combinations of each 2x2 spatial block,
scaled by 0.5.

Sharding: pure data parallel — one batch sample per NeuronCore (8 cores).

All device I/O is fp16 (host casts; the x0.5 subband scale is applied
during the host-side fp16 -> fp32 upcast): 8.4 MB in + 8.4 MB out per
core.

DMA model measured on this part (SPMD, all 8 cores streaming):
  - descriptors from all queues funnel into 16 per-engine FIFOs and
    are processed in ARRIVAL order at ~25 GB/s/engine (~425 GB/s
    aggregate = this core's HBM share), so the GLOBAL issue order
    is the transfer schedule;
  - SWDGE (GpSimd Q0) generates descriptors in bulk; each HWDGE
    ring (SP Q1 / ACT Q10) trickles at ~28 ns/descriptor — an HWDGE
    load issued BEFORE the SWDGE flood lands ~11-13 us, later ones
    crawl; never head an HWDGE queue with the 256 B-row weight load
    (128 tiny descriptors = 3.6 us of generator time);
  - at most ~8 outstanding SWDGE dma_starts before a framework
    DRAIN stalls the GpSimd stream (later store issues just wait);
  - a dma_start on a backed-up queue stalls its ENGINE 1-2 us, so
    all ScalarE copies are emitted at higher priority than any
    store issue.

The critical chain is VectorE (30.5 us of butterfly work): B0.top
is therefore the first-landing tile (Q1 head), and kc0 (PE's first
food) rides Q10's head. Everything else streams on Q0 in
consumption order; B1 lands last and fills VectorE's tail.

Work split by W-halves across two compute paths:

Path A (W cols 0:4096) — TensorE + ScalarE + VectorE, 8 units of
  128 rows x 2048 cols:
  - TensorE: row (H) butterfly as matmul with a fixed 128x128 +/-1
    fp16 weight (PSUM p0:63 = top+bot, p64:127 = top-bot);
  - ScalarE (ACT): PSUM -> SBUF copy, fp32 -> fp16;
  - VectorE: column (W) butterfly into per-kc [128, 2048] sum/diff
    tiles -> 4 merged [64 row, 4 KB] stores per kc.

Path B (W cols 4096:8192) — VectorE only, 2 units of 128 row-pairs:
  8-op butterfly (W-first: the first two mids need only `top`).

All tiles are resident in SBUF (~193 KB/partition, no reuse).
Bacc is built with num_devices=1: no collectives needed.
"""

import sys

if "/opt/trn_rl_repo" not in sys.path:
    sys.path.insert(0, "/opt/trn_rl_repo")

import numpy as np

B, H, W, C = 8, 512, 512, 16
N_CORES = 8
HO, WO = H // 2, W // 2  # 256, 256
ROW = W * C  # 8192 elements per input row
OROW = WO * C  # 4096 elements per output row

_CACHE = {}


def _haar_weight():
    """lhsT [k, m]: matmul computes out[m, n] = sum_k w[k, m] x[k, n]."""
    w = np.zeros((128, 128), dtype=np.float16)
    for m in range(64):
        w[2 * m, m] = 1.0
        w[2 * m + 1, m] = 1.0
        w[2 * m, 64 + m] = 1.0
        w[2 * m + 1, 64 + m] = -1.0
    return w


def _build():
    import concourse.bacc as bacc
    import concourse.mybir as mybir
    import concourse.tile as tile

    fp32 = mybir.dt.float32
    fp16 = mybir.dt.float16

    nc = bacc.Bacc(
        "TRN2", target_bir_lowering=False, debug=False, num_devices=1
    )
    x = nc.dram_tensor("x", (H, ROW), fp16, kind="ExternalInput")
    wdram = nc.dram_tensor("w", (128, 128), fp16, kind="ExternalInput")
    outs = {
        name: nc.dram_tensor(name, (HO, OROW), fp16, kind="ExternalOutput")
        for name in ("LL", "LH", "HL", "HH")
    }

    xq = x.rearrange("(q t) m -> q t m", t=2)  # [pair, row-parity, cols]

    HALF = ROW // 2  # 4096: A path covers cols 0:HALF, B path HALF:ROW
    AW = 2048  # A unit width (input cols); 4 matmuls of 512
    MM_N = 512  # one fp32 matmul / PSUM bank

    with tile.TileContext(nc) as tc:
        with (
            tc.tile_pool(name="main", bufs=1) as pool,
            tc.tile_pool(name="psum", bufs=2, space="PSUM") as psum,
        ):
            wt = pool.tile([128, 128], fp16, tag="wt")

            # ---- tiles ----
            tops = {}
            bots = {}
            for pc in range(2):
                tops[pc] = pool.tile(
                    [128, HALF], fp16, tag=f"top{pc}", name=f"top{pc}"
                )
                bots[pc] = pool.tile(
                    [128, HALF], fp16, tag=f"bot{pc}", name=f"bot{pc}"
                )
            # kc=0: two half tiles (Q10's first instructions);
            # kc=1,2,3: one [128, 4096] tile each on SWDGE.
            xth = {}
            for g in range(2):
                xth[g] = pool.tile(
                    [128, AW], fp16, tag=f"xt0{g}", name=f"xt0{g}"
                )
            xtf = {}
            for kc in range(1, 4):
                xtf[kc] = pool.tile(
                    [128, HALF], fp16, tag=f"xt{kc}", name=f"xt{kc}"
                )

            def a_src(kc, g):
                if kc == 0:
                    return xth[g][:]
                return xtf[kc][:, g * AW : (g + 1) * AW]

            # ---- loads (global arrival order = issue order) ----
            # Exactly ONE head load per HWDGE ring: its descriptors
            # finish generating (~28 ns each) before the SWDGE flood
            # arrives (~11.8 us), so it lands ~13 us. A second head
            # load would generate too late and get jumped by the flood.
            # Q1 head: B0.top, Q10 head: B0.bot — all four B0 mids
            # (VectorE's critical-chain start) are ready by ~13.2 us.
            # PE is not critical (ends ~32 vs DVE ~44), so kc0 rides
            # Q0's early slots instead of an HWDGE head.
            nc.sync.dma_start(tops[0][:], xq[slice(0, 128), 0, HALF:ROW])
            nc.scalar.dma_start(bots[0][:], xq[slice(0, 128), 1, HALF:ROW])
            # Q0 (SWDGE): wt (tiny, bulk-generated), then consumption
            # order — kc0, kc1..3, B1.
            nc.gpsimd.dma_start(wt[:], wdram[:])
            nc.gpsimd.dma_start(xth[0][:], x[0:128, 0:AW])
            nc.gpsimd.dma_start(xth[1][:], x[0:128, AW : 2 * AW])
            for kc in range(1, 4):
                nc.gpsimd.dma_start(
                    xtf[kc][:], x[kc * 128 : (kc + 1) * 128, 0:HALF]
                )
            nc.gpsimd.dma_start(tops[1][:], xq[slice(128, 256), 0, HALF:ROW])
            nc.gpsimd.dma_start(bots[1][:], xq[slice(128, 256), 1, HALF:ROW])

            # ---- B0 mids: highest DVE priority ----
            mids = {}
            for pc in range(2):
                for mt in ("t1", "t2", "u1", "u2"):
                    mids[(pc, mt)] = pool.tile(
                        [128, HALF // 2],
                        fp16,
                        tag=f"m{mt}{pc}",
                        name=f"m{mt}{pc}",
                    )

            def emit_b_mids(pc):
                tv = tops[pc][:].rearrange("p (w u c) -> p w u c", u=2, c=C)
                bv = bots[pc][:].rearrange("p (w u c) -> p w u c", u=2, c=C)
                a, b = tv[:, :, 0, :], tv[:, :, 1, :]
                c_, d = bv[:, :, 0, :], bv[:, :, 1, :]
                m = lambda mt: mids[(pc, mt)][:].rearrange(
                    "p (w c) -> p w c", c=C
                )
                # top-only ops first: they unblock as soon as `top` lands
                nc.vector.tensor_add(m("t1"), a, b)
                nc.vector.tensor_sub(m("u1"), a, b)
                nc.vector.tensor_add(m("t2"), c_, d)
                nc.vector.tensor_sub(m("u2"), c_, d)

            emit_b_mids(0)

            # ---- A units in land order (no stores yet: ScalarE's
            # copies must outrank every store issue) ----
            sums = {}
            diffs = {}
            for kc in range(4):
                sums[kc] = pool.tile(
                    [128, AW], fp16, tag=f"s{kc}", name=f"s{kc}"
                )
                diffs[kc] = pool.tile(
                    [128, AW], fp16, tag=f"d{kc}", name=f"d{kc}"
                )

            def emit_a_unit(kc, g):
                xt = a_src(kc, g)
                ps = psum.tile([128, AW], fp32)
                for j in range(AW // MM_N):
                    lo = j * MM_N
                    nc.tensor.matmul(
                        ps[:, lo : lo + MM_N],
                        wt[:],
                        xt[:, lo : lo + MM_N],
                        start=True,
                        stop=True,
                    )
                sb = pool.tile([128, AW], fp16, tag=f"sb{kc}{g}")
                nc.scalar.copy(sb[:], ps[:])  # ACT: PSUM -> SBUF, fp32->fp16
                sv_in = sb[:].rearrange("p (w u c) -> p w u c", u=2, c=C)
                ev, od = sv_in[:, :, 0, :], sv_in[:, :, 1, :]
                half = slice(g * (AW // 2), (g + 1) * (AW // 2))
                sv = sums[kc][:, half].rearrange("p (w c) -> p w c", c=C)
                dv = diffs[kc][:, half].rearrange("p (w c) -> p w c", c=C)
                nc.vector.tensor_add(sv, ev, od)
                nc.vector.tensor_sub(dv, ev, od)

            for kc in range(4):
                emit_a_unit(kc, 0)
                emit_a_unit(kc, 1)

            # ---- A stores (after all copies in priority) ----
            def emit_a_stores(kc, rings):
                rs = slice(kc * 64, (kc + 1) * 64)
                cols = slice(0, AW)
                r0, r1, r2, r3 = rings
                r0.dma_start(outs["LL"][rs, cols], sums[kc][0:64, :])
                r1.dma_start(outs["HL"][rs, cols], sums[kc][64:128, :])
                r2.dma_start(outs["LH"][rs, cols], diffs[kc][0:64, :])
                r3.dma_start(outs["HH"][rs, cols], diffs[kc][64:128, :])

            emit_a_stores(0, (nc.sync,) * 4)
            emit_a_stores(1, (nc.scalar,) * 4)
            emit_a_stores(2, (nc.sync,) * 4)
            emit_a_stores(3, (nc.gpsimd,) * 4)

            # ---- B outs: fill DVE gaps ----
            def emit_b_outs(pc, rings):
                qs = slice(pc * 128, (pc + 1) * 128)
                oc = slice(HALF // 2, OROW)
                WQ = HALF // (2 * C)
                for (name, i0, i1, op), ring in zip(
                    (
                        ("LL", "t1", "t2", "add"),
                        ("HL", "t1", "t2", "sub"),
                        ("LH", "u1", "u2", "add"),
                        ("HH", "u1", "u2", "sub"),
                    ),
                    rings,
                ):
                    ot = pool.tile([128, WQ, C], fp16, tag=f"o{name}{pc}")
                    a0 = mids[(pc, i0)][:].rearrange("p (w c) -> p w c", c=C)
                    a1 = mids[(pc, i1)][:].rearrange("p (w c) -> p w c", c=C)
                    if op == "add":
                        nc.vector.tensor_add(ot[:], a0, a1)
                    else:
                        nc.vector.tensor_sub(ot[:], a0, a1)
                    ring.dma_start(
                        outs[name][qs, oc],
                        ot[:].rearrange("p w c -> p (w c)"),
                    )

            # B0: LL/HL on HWDGE rings, LH/HH ride Q0 after its loads;
            # B1: LL/HL on HWDGE, LH/HH on Q0's free tail.
            emit_b_outs(0, (nc.sync, nc.scalar, nc.sync, nc.scalar))
            emit_b_mids(1)
            emit_b_outs(1, (nc.gpsimd, nc.gpsimd, nc.sync, nc.gpsimd))

    nc.compile()
    return nc


def _get_nc():
    if "nc" not in _CACHE:
        _CACHE["nc"] = _build()
    return _CACHE["nc"]


def _in_maps(x):
    w = _haar_weight()
    xh = np.asarray(x, dtype=np.float16)
    return [
        {"x": np.ascontiguousarray(xh[i].reshape(H, ROW)), "w": w}
        for i in range(B)
    ]


def kernel(x):
    from concourse.bass_utils import run_bass_kernel_spmd

    x = np.asarray(x, dtype=np.float32)
    assert x.shape == (B, H, W, C), x.shape

    nc = _get_nc()
    try:
        res = run_bass_kernel_spmd(nc, _in_maps(x), list(range(N_CORES)))
    except Exception:
        # transient NRT device errors have been observed right after
        # compile; one retry has always succeeded
        res = run_bass_kernel_spmd(nc, _in_maps(x), list(range(N_CORES)))

    out = []
    for name in ("LL", "LH", "HL", "HH"):
        sub = np.stack(
            [res.results[i][name].reshape(HO, WO, C) for i in range(B)],
            axis=0,
        )
        out.append(sub.astype(np.float32) * np.float32(0.5))
    return tuple(out)
